# revision 1
# baseline (speedup 1.0000x reference)
"""Per-pixel dynamic-filter 5x5 convolution (KPN-style) on 8 TRN2 NeuronCores.

Math: out[b,h,w] = sum_{di,dj,c} img[b, h+di-2, w+dj-2, c] * filts[b, h, w, (di*5+dj)*3+c]
Shapes: img [4,512,512,3] f32, filts [4,512,512,75] f32 -> out [4,512,512] f32.

Strategy (pure data parallel, no cross-core comms):
  - 8 shards = (batch b in 0..3) x (H half in 0..1); each core owns a
    [256, 512] output slab (2 fused 128-row h-tiles).
  - Host prep (per core): img padded + transposed to [h', c, x] fp16; filts
    transposed to [p, di, dj, c, ht, w] fp16 so each (di,dj) group of six
    (c,ht) planes sits at uniform stride.
  - On-chip per di: one img tile [p][c][ht][x520] (rows DMA'd at offset di)
    plus a one-element-x-shifted copy (ACT) so odd-dj operands stay
    4B-aligned. One DVE tensor_tensor per (di,dj) computes all six (c,ht)
    product planes in a single FD=3072 fp16 2x-mode instruction (25 TTs
    total). The TensorEngine accumulates the planes into two fp32 PSUM
    banks via identity matmuls; ACT evicts, DMA out.
  - Dummy-matmul warmup inside the first DMA shadow lifts the PE HAM
    clock throttle; per-di chunking puts the first TT ~6us in.
"""

import sys

sys.path.insert(0, "/opt/trn_rl_repo")

import numpy as np

from concourse import bass, bacc, mybir
from concourse.tile import TileContext
from concourse.bass_utils import run_bass_kernel_spmd

B, H, W, C = 4, 512, 512, 3
K = 5
KK = K * K * C  # 75
N_CORES = 8
HSH = H // 2  # 256 rows per shard
XP = W + 6  # img DRAM x extent: w in [-2, 516) -> x = w+2 in [0, 518)
XT = XP  # x extent in SBUF img tiles (even, so the (c,ht) plane stride stays 4B-aligned)
IMG_FREE = C * XP  # 1554 per padded DRAM img row
N_HT = HSH // 128  # 2 h-tiles per shard, fused
N_WARMUP_MM = 10

_F16 = mybir.dt.float16
_F32 = mybir.dt.float32

_NC = None


def build_nc():
    """Build the single-core Bass program (identical on all 8 cores)."""
    nc = bacc.Bacc("TRN2")
    NP = C * N_HT  # (c,ht) planes per (di,dj)
    img_d = nc.declare_dram_parameter("img", [HSH + 4, C, XP], _F16, isOutput=False)
    filts_d = nc.declare_dram_parameter(
        "filts", [128, K, K, C, N_HT, W], _F16, isOutput=False
    )
    ident_d = nc.declare_dram_parameter("ident", [128, 128], _F16, isOutput=False)
    out_d = nc.declare_dram_parameter("out", [HSH, W], _F32, isOutput=True)

    with TileContext(nc) as tc:
        with (
            tc.tile_pool(name="const", bufs=1) as constp,
            tc.tile_pool(name="imgp", bufs=3) as imgp,
            tc.tile_pool(name="filtp", bufs=3) as filtp,
            tc.tile_pool(name="prodp", bufs=3) as prodp,
            tc.tile_pool(name="outp", bufs=2) as outp,
            tc.tile_pool(name="psump", bufs=1, space="PSUM") as psump,
            tc.tile_pool(name="wpsump", bufs=1, space="PSUM") as wpsump,
        ):
            id_t = constp.tile([128, 128], _F16)
            nc.sync.dma_start(out=id_t[:], in_=ident_d[:])

            # PE warmup: dummy matmuls in the first DMAs' shadow lift HAM.
            wsrc = constp.tile([128, 512], _F16, tag="wsrc")
            nc.gpsimd.memset(wsrc[:], 0.0)
            wps = wpsump.tile([128, 512], _F32)
            for _ in range(N_WARMUP_MM):
                nc.tensor.matmul(wps[:], wsrc[:, :128], wsrc[:], start=True, stop=True)

            psum_t = [
                psump.tile([128, W], _F32, tag=f"ps{ht}", name=f"ps{ht}")
                for ht in range(N_HT)
            ]

            for di in range(K):
                # img tile layout [p][c][ht][x:XT]; plane k = N_HT*c + ht at
                # uniform stride XT. Rows at partition offset di.
                t0 = imgp.tile([128, C, N_HT, XT], _F16, tag="img0", name=f"img0_{di}")
                for ht in range(N_HT):
                    nc.sync.dma_start(
                        out=t0[:, :, ht, :XP],
                        in_=img_d[ht * 128 + di : ht * 128 + di + 128, :, :],
                    )
                # x-shifted-by-one copy keeps odd-dj operands 4B-aligned
                t1 = imgp.tile([128, C, N_HT, XT], _F16, tag="img1", name=f"img1_{di}")
                fl0 = t0[:].rearrange("p c t x -> p (c t x)")
                fl1 = t1[:].rearrange("p c t x -> p (c t x)")
                nfree = C * N_HT * XT
                nc.scalar.copy(out=fl1[:, 0 : nfree - 1], in_=fl0[:, 1:nfree])
                imgs = {0: t0, 1: t1}

                # filts for this di: [p][dj][c][ht][w], contiguous per
                # partition; split into two sub-DMAs for earlier first-use.
                ft = filtp.tile([128, K, C, N_HT, W], _F16, tag="ft", name=f"ft{di}")
                nc.sync.dma_start(out=ft[:, :2], in_=filts_d[:, di, :2])
                nc.sync.dma_start(out=ft[:, 2:], in_=filts_d[:, di, 2:])

                for dj in range(K):
                    q = dj & 1
                    x0 = dj - q
                    p_t = prodp.tile([128, NP, W], _F16, tag="pt", name=f"pt{di}{dj}")
                    src = imgs[q][:].rearrange("p c t x -> p (c t) x")
                    nc.vector.tensor_tensor(
                        p_t[:],
                        src[:, :, x0 : x0 + W],
                        ft[:, dj].rearrange("p c t w -> p (c t) w"),
                        mybir.AluOpType.mult,
                    )
                    first = di == 0 and dj == 0
                    last = di == K - 1 and dj == K - 1
                    for k in range(NP):
                        ht = k % N_HT
                        nc.tensor.matmul(
                            psum_t[ht][:],
                            id_t[:],
                            p_t[:, k, :],
                            start=(first and k < N_HT),
                            stop=(last and k >= NP - N_HT),
                        )

            for ht in range(N_HT):
                o_t = outp.tile([128, W], _F32, tag="ot", name=f"ot{ht}")
                nc.scalar.copy(out=o_t[:], in_=psum_t[ht][:])
                nc.sync.dma_start(out=out_d[ht * 128 : (ht + 1) * 128, :], in_=o_t[:])

    nc.compile()
    return nc


def get_nc():
    global _NC
    if _NC is None:
        _NC = build_nc()
    return _NC


def prepare_in_maps(img_stack: np.ndarray, filts: np.ndarray):
    """Shard + reformat FULL fp32 inputs into per-core fp16 input maps."""
    ident = np.eye(128, dtype=np.float16)
    in_maps = []
    for core in range(N_CORES):
        b, hh = divmod(core, 2)
        h0 = hh * HSH
        # img: pad h by 2 each side, w by 2 left / 4 right -> [516, 518, 3]
        padded = np.pad(img_stack[b], ((2, 2), (2, XP - W - 2), (0, 0)))
        shard = padded[h0 : h0 + HSH + 4]  # rows h0-2 .. h0+258
        img_p = np.ascontiguousarray(shard.transpose(0, 2, 1)).astype(np.float16)
        # filts -> [p, di, dj, c, ht, w]
        f = filts[b, h0 : h0 + HSH].reshape(N_HT, 128, W, K, K, C)
        filts_p = np.ascontiguousarray(f.transpose(1, 3, 4, 5, 0, 2)).astype(
            np.float16
        )
        in_maps.append({"img": img_p, "filts": filts_p, "ident": ident})
    return in_maps


def assemble_out(results) -> np.ndarray:
    out = np.empty((B, H, W), dtype=np.float32)
    for core in range(N_CORES):
        b, hh = divmod(core, 2)
        out[b, hh * HSH : (hh + 1) * HSH, :] = results[core]["out"]
    return out


def kernel(img_stack: np.ndarray, filts: np.ndarray) -> np.ndarray:
    nc = get_nc()
    in_maps = prepare_in_maps(img_stack, filts)
    res = run_bass_kernel_spmd(nc, in_maps, list(range(N_CORES)))
    return assemble_out(res.results)

